# revision 21
# baseline (speedup 1.0000x reference)
"""ConvAConnect TRN2 kernel: per-sample noisy-weight 3x3 conv, data-parallel over 8 cores.

Z[b] = conv2d_valid(X[b], W * Werr[loc_id[b]]) + bias * Berr[loc_id[b]]

Shapes: X[32,64,64,64] f32, W[3,3,64,128], bias[128], Werr[1000,3,3,64,128],
Berr[1000,128], loc_id[32] i32 -> Z[32,62,62,128] f32.

Strategy: shard batch (4 samples/core). Per the sharding hint, the per-sample
noisy weights memW = W*Werr[loc_id] and membias = bias*Berr[loc_id] are formed
host-side and sharded with the batch.

Device kernel per sample (fp16 operands, f32 PSUM accumulate):
  - ONE packed host tensor per sample: [128, 768+4104] fp16 =
    [ noisy weights | stacked X^T ], where the stack is
    [X^T ; X^T shifted 64 pixels (one grid row)]. The 9 conv taps become 6
    K=128 matmuls per 512-pixel output chunk: 3 row-pair blocks (fh 0+1,
    fw j) read the stack at offset j, and 3 single blocks (fh 2, fw j,
    lower 64 weight rows zero) read at offset 128+j.
  - Packing matters because each DMA descriptor is sliced across the 16
    shared physical DMA engines and completes (semaphore-wise) only when
    the SLOWEST engine finishes; at startup one engine serves descriptor
    waves serially (~1us each), so chunk 0's start time is simply (number
    of descriptor waves ahead of it) x ~1us. One packed head descriptor
    carries weights + the first X columns together.
  - The engines also bound total traffic (~220 GB/s reads): the single
    stack (X twice, ~9 MB/core total) keeps the kernel PE-bound; schemes
    that duplicate X further (e.g. pairing the third conv row via a
    shift-1 stack) go DMA-bound and lose.
  - Output grid is 62 rows x 64 cols (2 junk columns keep width-64
    alignment); junk columns are dropped on host. PSUM drains (VectorE
    tensor_scalar_add) fuse the per-sample bias add and emit fp16 into a
    [cout, grid] zbuf; host does the final transpose.
  - Warm-up matmuls on a zeroed scratch tile ramp the PE p-state (DVFS)
    while the head descriptor streams in.
"""

import sys
import numpy as np

for _p in ("/opt/trn_rl_repo", "/root/.axon_site"):
    if _p not in sys.path:
        sys.path.insert(0, _p)

N_CORES = 8
B = 32
PER_CORE = B // N_CORES
H = Wd = 64
CIN = 64
COUT = 128
HO = WO = 62
GRID = HO * 64          # 62 rows x 64 cols (2 junk cols/row)
XTL = 4104              # stacked X^T free length (max read 4097)
NMM = 6                 # matmuls per chunk
WCAT = NMM * COUT       # 3 pair blocks | 3 single blocks (lower rows zero)
NCHUNK = 512            # output-grid pixels per PSUM chunk (8 grid rows)
NCHUNKS = 8             # 7 full chunks + 1 of 384
# Chunks >= QCHUNK use a 5-matmul schedule: taps (2,0)+(2,1) pair via a
# second stack [X^T<<128 ; X^T<<129] covering just those chunks' columns.
# This trades spare DMA-engine bandwidth for PE passes (the bottleneck).
QCHUNK = 3
QOFF = QCHUNK * NCHUNK  # first grid column covered by the q stack
QW = 2 * COUT           # q-pair block | q-single block (lower rows zero)
QXL = GRID - QOFF + 4   # q stack free length (max read GRID-QOFF+2)
TLEN = WCAT + XTL + QW + QXL  # packed: [mw | stack | mwq | q stack]
HEAD = WCAT + 650       # head descriptor: weights + chunk-0 columns
MID = (HEAD + TLEN) // 2

_compiled = {}


def _build():
    import concourse.mybir as mybir
    import concourse.tile as tile
    from concourse import bacc

    f32 = mybir.dt.float32
    f16 = mybir.dt.float16

    nc = bacc.Bacc("TRN2", target_bir_lowering=False, debug=False)

    xw_in = nc.dram_tensor("xw", [PER_CORE, 128, TLEN], f16, kind="ExternalInput")
    mb_in = nc.dram_tensor("mb", [COUT, PER_CORE], f32, kind="ExternalInput")
    z_out = nc.dram_tensor("z", [PER_CORE, 128, GRID], f16, kind="ExternalOutput")

    with tile.TileContext(nc) as tc:
        with (
            tc.tile_pool(name="const", bufs=1) as const,
            tc.tile_pool(name="xwpool", bufs=3) as xwpool,
            tc.tile_pool(name="zpool", bufs=3) as zpool,
            tc.tile_pool(name="psmm", bufs=4, space="PSUM") as psmm,
            tc.tile_pool(name="psw", bufs=1, space="PSUM") as psw,
        ):
            mb_all = const.tile([COUT, PER_CORE], f32, tag="mb")

            # PE warm-up: throwaway matmuls on a zeroed scratch tile ramp
            # the Tensor engine p-state while the first loads are in flight
            warm = const.tile([128, NCHUNK], f16, tag="warm")
            nc.gpsimd.memset(warm[:], 0.0)
            pw = psw.tile([128, NCHUNK], f32, tag="pw")
            for _ in range(5):
                nc.tensor.matmul(
                    pw[:], warm[:, 0:COUT], warm[:], start=True, stop=True
                )

            def load_sample(b, split):
                """DMA the packed [weights | stacked X] tile for sample b."""
                t = xwpool.tile([128, TLEN], f16, tag="xw")
                if split:
                    # head = weights + chunk-0 columns in ONE descriptor
                    # wave; the remainder follows in two balanced pieces
                    nc.sync.dma_start(t[:, 0:HEAD], xw_in[b][:, 0:HEAD])
                    nc.gpsimd.dma_start(mb_all[:], mb_in[:])
                    nc.scalar.dma_start(t[:, HEAD:MID], xw_in[b][:, HEAD:MID])
                    nc.sync.dma_start(t[:, MID:TLEN], xw_in[b][:, MID:TLEN])
                else:
                    e1, e2 = (nc.sync, nc.scalar) if b % 2 else (nc.scalar, nc.sync)
                    e1.dma_start(t[:, 0:MID], xw_in[b][:, 0:MID])
                    e2.dma_start(t[:, MID:TLEN], xw_in[b][:, MID:TLEN])
                return t

            samples = [load_sample(0, True), load_sample(1, False)]
            for b in range(PER_CORE):
                t = samples[b]
                if b + 2 < PER_CORE:
                    samples.append(load_sample(b + 2, False))
                mw = t[:, 0:WCAT]
                xts = t[:, WCAT : WCAT + XTL]
                mwq = t[:, WCAT + XTL : WCAT + XTL + QW]
                xq = t[:, WCAT + XTL + QW : TLEN]

                zbuf = zpool.tile([128, GRID], f16, tag="zbuf")

                for c in range(NCHUNKS):
                    base = c * NCHUNK
                    ncols = min(NCHUNK, GRID - base)
                    pc = psmm.tile([128, NCHUNK], f32, tag="pc")
                    # taps (0,j)+(1,j): K=128 row pairs from the stack
                    for j in range(3):
                        nc.tensor.matmul(
                            pc[:, :ncols],
                            mw[:, j * COUT : (j + 1) * COUT],
                            xts[:, base + j : base + j + ncols],
                            start=(j == 0),
                            stop=False,
                        )
                    if c >= QCHUNK:
                        # taps (2,0)+(2,1): one K=128 pair from the q stack
                        nc.tensor.matmul(
                            pc[:, :ncols],
                            mwq[:, 0:COUT],
                            xq[:, base - QOFF : base - QOFF + ncols],
                            start=False,
                            stop=False,
                        )
                        # tap (2,2): K=128 with zero lower weight rows
                        nc.tensor.matmul(
                            pc[:, :ncols],
                            mwq[:, COUT:QW],
                            xq[:, base - QOFF + 2 : base - QOFF + 2 + ncols],
                            start=False,
                            stop=True,
                        )
                    else:
                        # taps (2,j): K=128 with zero lower weight rows
                        for j in range(3):
                            nc.tensor.matmul(
                                pc[:, :ncols],
                                mw[:, (3 + j) * COUT : (4 + j) * COUT],
                                xts[:, base + 128 + j : base + 128 + j + ncols],
                                start=False,
                                stop=(j == 2),
                            )
                    # drain PSUM -> zbuf fused with the per-sample bias add;
                    # all drains on VectorE keeps ScalarE a pure DMA engine
                    nc.vector.tensor_scalar_add(
                        zbuf[:, base : base + ncols],
                        pc[:, :ncols],
                        mb_all[:, b : b + 1],
                    )
                    # last sample ships in pieces as chunks drain so the
                    # final DMA tail is only the 384-col last chunk
                    ZCUTS = {1: (0, 992), 3: (992, 1984), 5: (1984, 2976),
                             6: (2976, 3584), 7: (3584, GRID)}
                    if b == PER_CORE - 1 and c in ZCUTS:
                        lo, hi = ZCUTS[c]
                        eng = (nc.sync, nc.scalar)[c % 2]
                        eng.dma_start(z_out[b][:, lo:hi], zbuf[:, lo:hi])

                # ship the sample (host does the final transpose); the
                # last sample already shipped in pieces inline above
                if b < PER_CORE - 1:
                    eng = (nc.scalar, nc.sync, nc.gpsimd)[b]
                    eng.dma_start(z_out[b], zbuf[:])

    nc.compile()
    return nc


def _get_nc():
    if "nc" not in _compiled:
        _compiled["nc"] = _build()
    return _compiled["nc"]


def _prep_inputs(X, W, bias, Werr, Berr, loc_id):
    """Host-side shard/layout prep. Returns per-core in_maps."""
    X = np.asarray(X, dtype=np.float32)
    W = np.asarray(W, dtype=np.float32)
    bias = np.asarray(bias, dtype=np.float32)
    Werr = np.asarray(Werr, dtype=np.float32)
    Berr = np.asarray(Berr, dtype=np.float32)
    loc_id = np.asarray(loc_id)

    # X^T: [B, CIN, H*W] zero-padded, fp16
    xsrc = XTL + 64
    xt = np.zeros((B, CIN, xsrc), dtype=np.float16)
    xt[:, :, : H * Wd] = X.transpose(0, 3, 1, 2).reshape(B, CIN, H * Wd)

    # memW = W * Werr[loc_id], laid out as [128, 768]:
    #   pair block j: rows = [memW[0, j, cin, :]; memW[1, j, cin, :]]
    #   single block j: rows = [memW[2, j, cin, :]; zeros]
    def cat_blocks(w):
        lead = w.shape[:-4]
        out = np.zeros(lead + (128, WCAT), dtype=np.float16)
        # [..., fh2, fw, cin, cout] -> [..., fw, fh2*cin, cout]
        pair = np.moveaxis(w[..., 0:2, :, :, :], -3, -4).reshape(
            lead + (3, 128, COUT)
        )
        for j in range(3):
            out[..., :, j * COUT : (j + 1) * COUT] = pair[..., j, :, :]
            out[..., 0:64, (3 + j) * COUT : (4 + j) * COUT] = w[..., 2, j, :, :]
        return out

    mwcat = cat_blocks(W[None] * Werr[loc_id])   # [B, 128, 768] fp16
    mb = (bias[None] * Berr[loc_id]).astype(np.float32)  # [B, 128]

    # packed [mw | stack | mwq | q stack]: stack = [X^T ; X^T << 64],
    # q stack = [X^T << (QOFF+128) ; X^T << (QOFF+129)] for the 5-pass chunks
    memw = W[None] * Werr[loc_id]
    xw = np.empty((B, 128, TLEN), dtype=np.float16)
    xw[:, :, :WCAT] = mwcat
    a = WCAT
    xw[:, 0:64, a : a + XTL] = xt[:, :, 0:XTL]
    xw[:, 64:128, a : a + XTL] = xt[:, :, 64 : 64 + XTL]
    a += XTL
    xw[:, 0:64, a : a + COUT] = memw[:, 2, 0].astype(np.float16)
    xw[:, 64:128, a : a + COUT] = memw[:, 2, 1].astype(np.float16)
    xw[:, 0:64, a + COUT : a + QW] = memw[:, 2, 2].astype(np.float16)
    xw[:, 64:128, a + COUT : a + QW] = 0.0
    a += QW
    xw[:, 0:64, a:] = xt[:, :, QOFF + 128 : QOFF + 128 + QXL]
    xw[:, 64:128, a:] = xt[:, :, QOFF + 129 : QOFF + 129 + QXL]

    in_maps = []
    for i in range(N_CORES):
        s = slice(i * PER_CORE, (i + 1) * PER_CORE)
        in_maps.append(
            {
                "xw": np.ascontiguousarray(xw[s]),
                "mb": np.ascontiguousarray(mb[s].T),
            }
        )
    return in_maps


def _run(in_maps, trace=False, **kw):
    from concourse.bass_utils import run_bass_kernel_spmd

    nc = _get_nc()
    return run_bass_kernel_spmd(nc, in_maps, list(range(N_CORES)), trace=trace, **kw)


def _unshard(results):
    zb = np.concatenate([results[i]["z"] for i in range(N_CORES)], axis=0)
    # zb[b, cout, ho*64+wo] -> Z[b, ho, wo, cout]
    v = zb.astype(np.float32).reshape(B, COUT, HO, 64).transpose(0, 2, 3, 1)
    return np.ascontiguousarray(v[:, :, :WO, :])


def kernel(X, W, bias, Werr, Berr, loc_id):
    in_maps = _prep_inputs(X, W, bias, Werr, Berr, loc_id)
    res = _run(in_maps)
    return _unshard(res.results)


# revision 23
# speedup vs baseline: 1.0610x; 1.0610x over previous
"""ConvAConnect TRN2 kernel: per-sample noisy-weight 3x3 conv, data-parallel over 8 cores.

Z[b] = conv2d_valid(X[b], W * Werr[loc_id[b]]) + bias * Berr[loc_id[b]]

Shapes: X[32,64,64,64] f32, W[3,3,64,128], bias[128], Werr[1000,3,3,64,128],
Berr[1000,128], loc_id[32] i32 -> Z[32,62,62,128] f32.

Strategy: shard batch (4 samples/core). Per the sharding hint, the per-sample
noisy weights memW = W*Werr[loc_id] and membias = bias*Berr[loc_id] are formed
host-side and sharded with the batch.

Device kernel per sample (fp16 operands, f32 PSUM accumulate):
  - ONE packed host tensor per sample: [128, 768+4104] fp16 =
    [ noisy weights | stacked X^T ], where the stack is
    [X^T ; X^T shifted 64 pixels (one grid row)]. The 9 conv taps become 6
    K=128 matmuls per 512-pixel output chunk: 3 row-pair blocks (fh 0+1,
    fw j) read the stack at offset j, and 3 single blocks (fh 2, fw j,
    lower 64 weight rows zero) read at offset 128+j.
  - Packing matters because each DMA descriptor is sliced across the 16
    shared physical DMA engines and completes (semaphore-wise) only when
    the SLOWEST engine finishes; at startup one engine serves descriptor
    waves serially (~1us each), so chunk 0's start time is simply (number
    of descriptor waves ahead of it) x ~1us. One packed head descriptor
    carries weights + the first X columns together.
  - The engines also bound total traffic (~220 GB/s reads): the single
    stack (X twice, ~9 MB/core total) keeps the kernel PE-bound; schemes
    that duplicate X further (e.g. pairing the third conv row via a
    shift-1 stack) go DMA-bound and lose.
  - Output grid is 62 rows x 64 cols (2 junk columns keep width-64
    alignment); junk columns are dropped on host. PSUM drains (VectorE
    tensor_scalar_add) fuse the per-sample bias add and emit fp16 into a
    [cout, grid] zbuf; host does the final transpose.
  - Warm-up matmuls on a zeroed scratch tile ramp the PE p-state (DVFS)
    while the head descriptor streams in.
"""

import sys
import numpy as np

for _p in ("/opt/trn_rl_repo", "/root/.axon_site"):
    if _p not in sys.path:
        sys.path.insert(0, _p)

N_CORES = 8
B = 32
PER_CORE = B // N_CORES
H = Wd = 64
CIN = 64
COUT = 128
HO = WO = 62
GRID = HO * 64          # 62 rows x 64 cols (2 junk cols/row)
XTL = 4104              # stacked X^T free length (max read 4097)
NMM = 6                 # matmuls per chunk
WCAT = NMM * COUT       # 3 pair blocks | 3 single blocks (lower rows zero)
NCHUNK = 512            # output-grid pixels per PSUM chunk (8 grid rows)
NCHUNKS = 8             # 7 full chunks + 1 of 384
# Chunks >= QCHUNK use a 5-matmul schedule: taps (2,0)+(2,1) pair via a
# second stack [X^T<<128 ; X^T<<129] covering just those chunks' columns.
# This trades spare DMA-engine bandwidth for PE passes (the bottleneck).
QCHUNK = 2
QOFF = QCHUNK * NCHUNK  # first grid column covered by the q stack
QW = 2 * COUT           # q-pair block | q-single block (lower rows zero)
QXL = GRID - QOFF + 4   # q stack free length (max read GRID-QOFF+2)
TLEN = WCAT + XTL + QW + QXL  # packed: [mw | stack | mwq | q stack]
HEAD = WCAT + 650       # head descriptor: weights + chunk-0 columns
MID = (HEAD + TLEN) // 2

_compiled = {}


def _build():
    import concourse.mybir as mybir
    import concourse.tile as tile
    from concourse import bacc

    f32 = mybir.dt.float32
    f16 = mybir.dt.float16

    nc = bacc.Bacc("TRN2", target_bir_lowering=False, debug=False)

    xw_in = nc.dram_tensor("xw", [PER_CORE, 128, TLEN], f16, kind="ExternalInput")
    mb_in = nc.dram_tensor("mb", [COUT, PER_CORE], f32, kind="ExternalInput")
    z_out = nc.dram_tensor("z", [PER_CORE, 128, GRID], f16, kind="ExternalOutput")

    with tile.TileContext(nc) as tc:
        with (
            tc.tile_pool(name="const", bufs=1) as const,
            tc.tile_pool(name="xwpool", bufs=3) as xwpool,
            tc.tile_pool(name="zpool", bufs=3) as zpool,
            tc.tile_pool(name="psmm", bufs=4, space="PSUM") as psmm,
            tc.tile_pool(name="psw", bufs=1, space="PSUM") as psw,
        ):
            mb_all = const.tile([COUT, PER_CORE], f32, tag="mb")

            # PE warm-up: throwaway matmuls on a zeroed scratch tile ramp
            # the Tensor engine p-state while the first loads are in flight
            warm = const.tile([128, NCHUNK], f16, tag="warm")
            nc.gpsimd.memset(warm[:], 0.0)
            pw = psw.tile([128, NCHUNK], f32, tag="pw")
            for _ in range(5):
                nc.tensor.matmul(
                    pw[:], warm[:, 0:COUT], warm[:], start=True, stop=True
                )

            def load_sample(b, split):
                """DMA the packed [weights | stacked X] tile for sample b."""
                t = xwpool.tile([128, TLEN], f16, tag="xw")
                if split:
                    # head = weights + chunk-0 columns in ONE descriptor
                    # wave. Later waves are ordered by first-use time:
                    # stack for chunks 1-3, then the q region (chunk 2+
                    # fifth/sixth taps), then the stack tail.
                    s2 = WCAT + 2100
                    qs = WCAT + XTL
                    nc.sync.dma_start(t[:, 0:HEAD], xw_in[b][:, 0:HEAD])
                    nc.gpsimd.dma_start(mb_all[:], mb_in[:])
                    nc.scalar.dma_start(t[:, HEAD:s2], xw_in[b][:, HEAD:s2])
                    nc.sync.dma_start(t[:, qs:TLEN], xw_in[b][:, qs:TLEN])
                    nc.scalar.dma_start(t[:, s2:qs], xw_in[b][:, s2:qs])
                else:
                    e1, e2 = (nc.sync, nc.scalar) if b % 2 else (nc.scalar, nc.sync)
                    e1.dma_start(t[:, 0:MID], xw_in[b][:, 0:MID])
                    e2.dma_start(t[:, MID:TLEN], xw_in[b][:, MID:TLEN])
                return t

            samples = [load_sample(0, True), load_sample(1, False)]
            for b in range(PER_CORE):
                t = samples[b]
                if b + 2 < PER_CORE:
                    samples.append(load_sample(b + 2, False))
                mw = t[:, 0:WCAT]
                xts = t[:, WCAT : WCAT + XTL]
                mwq = t[:, WCAT + XTL : WCAT + XTL + QW]
                xq = t[:, WCAT + XTL + QW : TLEN]

                zbuf = zpool.tile([128, GRID], f16, tag="zbuf")

                for c in range(NCHUNKS):
                    base = c * NCHUNK
                    ncols = min(NCHUNK, GRID - base)
                    pc = psmm.tile([128, NCHUNK], f32, tag="pc")
                    # taps (0,j)+(1,j): K=128 row pairs from the stack
                    for j in range(3):
                        nc.tensor.matmul(
                            pc[:, :ncols],
                            mw[:, j * COUT : (j + 1) * COUT],
                            xts[:, base + j : base + j + ncols],
                            start=(j == 0),
                            stop=False,
                        )
                    if c >= QCHUNK:
                        # taps (2,0)+(2,1): one K=128 pair from the q stack
                        nc.tensor.matmul(
                            pc[:, :ncols],
                            mwq[:, 0:COUT],
                            xq[:, base - QOFF : base - QOFF + ncols],
                            start=False,
                            stop=False,
                        )
                        # tap (2,2): K=128 with zero lower weight rows
                        nc.tensor.matmul(
                            pc[:, :ncols],
                            mwq[:, COUT:QW],
                            xq[:, base - QOFF + 2 : base - QOFF + 2 + ncols],
                            start=False,
                            stop=True,
                        )
                    else:
                        # taps (2,j): K=128 with zero lower weight rows
                        for j in range(3):
                            nc.tensor.matmul(
                                pc[:, :ncols],
                                mw[:, (3 + j) * COUT : (4 + j) * COUT],
                                xts[:, base + 128 + j : base + 128 + j + ncols],
                                start=False,
                                stop=(j == 2),
                            )
                    # drain PSUM -> zbuf fused with the per-sample bias add;
                    # all drains on VectorE keeps ScalarE a pure DMA engine
                    nc.vector.tensor_scalar_add(
                        zbuf[:, base : base + ncols],
                        pc[:, :ncols],
                        mb_all[:, b : b + 1],
                    )
                    # last sample ships in pieces as chunks drain so the
                    # final DMA tail is only the 384-col last chunk
                    ZCUTS = {1: (0, 992), 3: (992, 1984), 5: (1984, 2976),
                             6: (2976, 3584), 7: (3584, GRID)}
                    if b == PER_CORE - 1 and c in ZCUTS:
                        lo, hi = ZCUTS[c]
                        eng = (nc.sync, nc.scalar)[c % 2]
                        eng.dma_start(z_out[b][:, lo:hi], zbuf[:, lo:hi])

                # ship the sample (host does the final transpose); the
                # last sample already shipped in pieces inline above
                if b < PER_CORE - 1:
                    eng = (nc.scalar, nc.sync, nc.gpsimd)[b]
                    eng.dma_start(z_out[b], zbuf[:])

    nc.compile()
    return nc


def _get_nc():
    if "nc" not in _compiled:
        _compiled["nc"] = _build()
    return _compiled["nc"]


def _prep_inputs(X, W, bias, Werr, Berr, loc_id):
    """Host-side shard/layout prep. Returns per-core in_maps."""
    X = np.asarray(X, dtype=np.float32)
    W = np.asarray(W, dtype=np.float32)
    bias = np.asarray(bias, dtype=np.float32)
    Werr = np.asarray(Werr, dtype=np.float32)
    Berr = np.asarray(Berr, dtype=np.float32)
    loc_id = np.asarray(loc_id)

    # X^T: [B, CIN, H*W] zero-padded, fp16
    xsrc = XTL + 64
    xt = np.zeros((B, CIN, xsrc), dtype=np.float16)
    xt[:, :, : H * Wd] = X.transpose(0, 3, 1, 2).reshape(B, CIN, H * Wd)

    # memW = W * Werr[loc_id], laid out as [128, 768]:
    #   pair block j: rows = [memW[0, j, cin, :]; memW[1, j, cin, :]]
    #   single block j: rows = [memW[2, j, cin, :]; zeros]
    def cat_blocks(w):
        lead = w.shape[:-4]
        out = np.zeros(lead + (128, WCAT), dtype=np.float16)
        # [..., fh2, fw, cin, cout] -> [..., fw, fh2*cin, cout]
        pair = np.moveaxis(w[..., 0:2, :, :, :], -3, -4).reshape(
            lead + (3, 128, COUT)
        )
        for j in range(3):
            out[..., :, j * COUT : (j + 1) * COUT] = pair[..., j, :, :]
            out[..., 0:64, (3 + j) * COUT : (4 + j) * COUT] = w[..., 2, j, :, :]
        return out

    mwcat = cat_blocks(W[None] * Werr[loc_id])   # [B, 128, 768] fp16
    mb = (bias[None] * Berr[loc_id]).astype(np.float32)  # [B, 128]

    # packed [mw | stack | mwq | q stack]: stack = [X^T ; X^T << 64],
    # q stack = [X^T << (QOFF+128) ; X^T << (QOFF+129)] for the 5-pass chunks
    memw = W[None] * Werr[loc_id]
    xw = np.empty((B, 128, TLEN), dtype=np.float16)
    xw[:, :, :WCAT] = mwcat
    a = WCAT
    xw[:, 0:64, a : a + XTL] = xt[:, :, 0:XTL]
    xw[:, 64:128, a : a + XTL] = xt[:, :, 64 : 64 + XTL]
    a += XTL
    xw[:, 0:64, a : a + COUT] = memw[:, 2, 0].astype(np.float16)
    xw[:, 64:128, a : a + COUT] = memw[:, 2, 1].astype(np.float16)
    xw[:, 0:64, a + COUT : a + QW] = memw[:, 2, 2].astype(np.float16)
    xw[:, 64:128, a + COUT : a + QW] = 0.0
    a += QW
    xw[:, 0:64, a:] = xt[:, :, QOFF + 128 : QOFF + 128 + QXL]
    xw[:, 64:128, a:] = xt[:, :, QOFF + 129 : QOFF + 129 + QXL]

    in_maps = []
    for i in range(N_CORES):
        s = slice(i * PER_CORE, (i + 1) * PER_CORE)
        in_maps.append(
            {
                "xw": np.ascontiguousarray(xw[s]),
                "mb": np.ascontiguousarray(mb[s].T),
            }
        )
    return in_maps


def _run(in_maps, trace=False, **kw):
    from concourse.bass_utils import run_bass_kernel_spmd

    nc = _get_nc()
    return run_bass_kernel_spmd(nc, in_maps, list(range(N_CORES)), trace=trace, **kw)


def _unshard(results):
    zb = np.concatenate([results[i]["z"] for i in range(N_CORES)], axis=0)
    # zb[b, cout, ho*64+wo] -> Z[b, ho, wo, cout]
    v = zb.astype(np.float32).reshape(B, COUT, HO, 64).transpose(0, 2, 3, 1)
    return np.ascontiguousarray(v[:, :, :WO, :])


def kernel(X, W, bias, Werr, Berr, loc_id):
    in_maps = _prep_inputs(X, W, bias, Werr, Berr, loc_id)
    res = _run(in_maps)
    return _unshard(res.results)
